# revision 13
# baseline (speedup 1.0000x reference)
"""CRF forward (log-partition) kernel for Trainium2, 8 NeuronCores.

Algorithm: exp-space scaled forward recurrence (classic scaled HMM forward),
split into a forward and a backward half that run simultaneously and meet in
the middle — this halves the serial dependency chain (the kernel is bound by
per-step PE<->DVE roundtrip latency, not throughput).

    forward : p_k = d_k * (E^T p_{k-1}),  p_0 = exp(start) * d_0
    backward: v_t = d_t * (E v_{t+1}),    v_511 = exp(end) * d_511
    d_t = exp(emit_t - c),  E = exp(T),  c = fixed rescale constant
    logZ = S*c + ln( sum_j (E^T p_255)_j * (v_256)_j )

Both directions share each matmul: the stationary weight is
blockdiag(E, E^T) [128x128]; the state tile stacks [64 forward labels |
64 backward labels] on partitions with batch on the free dim. bf16 state /
weights (safe: the output is log-scale ~2379, so ~0.2% linear-space rounding
averages out to ~3e-5 relative error).

Sharding: batch 1024 -> 8 cores x 128; per core 2 interleaved wavefront
chains (batch halves) hide the PE<->DVE latency. Emissions are
pre-transposed on the host into DMA-contiguous per-chunk tiles, exp'd in
bulk on ACT, then re-homed to DVE (so the per-step muls carry no
cross-engine waits). Redundant per-matmul LDWEIGHTS are stripped
post-compile (the stationary weights never change mid-loop).
"""

import numpy as np
import ml_dtypes
from contextlib import ExitStack

import concourse.bass as bass
import concourse.bacc as bacc
import concourse.tile as tile
from concourse import mybir
from concourse.bass_utils import run_bass_kernel_spmd

# Problem constants (hardcoded per contract: shapes are fixed)
B, S, L = 1024, 512, 64
NCORES = 8
NCHAIN = 2            # wavefront chains per core (batch halves)
BPC = B // NCORES     # 128 batch per core
CB = BPC // NCHAIN    # 64 batch per chain = matmul free dim
TM = S // 2           # 256 wavefronts (fwd + bwd meet in the middle)
# Variable chunk schedule: small head chunks so the recurrence starts while
# the bulk still streams; 16-wavefront chunks in steady state.
CHUNKS = [4, 4, 8] + [16] * 15
assert sum(CHUNKS) == TM
C_NORM = 4.6466287    # per-step rescale constant (offline calibrated)

_CACHE: dict = {}


def _build_nc():
    f32 = mybir.dt.float32
    bf16 = mybir.dt.bfloat16
    nc = bacc.Bacc(None, target_bir_lowering=False)
    emt = nc.declare_dram_parameter(
        "emt", [NCHAIN, 128, TM, CB], f32, isOutput=False
    )
    wts = nc.declare_dram_parameter("wts", [128, 128], bf16, isOutput=False)
    cvec = nc.declare_dram_parameter("cvec", [128, 2], f32, isOutput=False)
    ish = nc.declare_dram_parameter("ish", [128, 64], bf16, isOutput=False)
    ones = nc.declare_dram_parameter("ones", [64, 1], f32, isOutput=False)
    outp = nc.declare_dram_parameter("out", [NCHAIN, CB], f32, isOutput=True)

    EXP = mybir.ActivationFunctionType.Exp
    LN = mybir.ActivationFunctionType.Ln
    COPY = mybir.ActivationFunctionType.Copy
    EMBUFS = 5

    with ExitStack() as ctx:
        tc = ctx.enter_context(tile.TileContext(nc))
        consts = ctx.enter_context(tc.tile_pool(name="consts", bufs=1))
        empool = ctx.enter_context(tc.tile_pool(name="em", bufs=EMBUFS))
        state = ctx.enter_context(tc.tile_pool(name="state", bufs=12))
        psum = ctx.enter_context(
            tc.tile_pool(name="psum", bufs=2, space=bass.MemorySpace.PSUM)
        )

        w_t = consts.tile([128, 128], bf16)
        cv_t = consts.tile([128, 2], f32)
        ish_t = consts.tile([128, 64], bf16)
        on_t = consts.tile([64, 1], f32)
        nc.sync.dma_start(out=w_t, in_=wts[:, :])
        nc.sync.dma_start(out=cv_t, in_=cvec[:, :])
        nc.sync.dma_start(out=ish_t, in_=ish[:, :])
        nc.sync.dma_start(out=on_t, in_=ones[:, :])

        # Warmups: make each engine observe the const DMAs up front so no
        # steady-state instruction needs more than one sem wait.
        aw = consts.tile([128, 2], f32, tag="actwarm")
        nc.scalar.activation(out=aw, in_=cv_t, func=COPY)
        dw = consts.tile([128, 1], f32, tag="dvewarm")
        nc.vector.tensor_copy(dw, cv_t[:, 0:1])
        ow = consts.tile([64, 1], f32, tag="oneswarm")
        nc.vector.tensor_copy(ow, on_t)
        wq = psum.tile([128, 2], f32, tag="warm", bufs=1)
        nc.tensor.matmul(wq[0:64, :], ish_t[:, 0:64], ish_t[:, 0:2], start=True, stop=True)
        # last warmup leaves the main stationary weights resident
        nc.tensor.matmul(wq, w_t, ish_t[:, 0:2], start=True, stop=True)

        s_cur = [None] * NCHAIN
        dts_hist: list[list] = []
        t0 = 0
        for j, kj in enumerate(CHUNKS):
            dds = []
            for x in range(NCHAIN):
                raw = empool.tile([128, 16, CB], f32, tag=f"raw{x}", name=f"raw{x}_{j}")
                nc.sync.dma_start(
                    out=raw[:, 0:kj, :], in_=emt[x, :, t0 : t0 + kj, :]
                )
                dt = empool.tile([128, 16, CB], bf16, tag=f"d{x}", name=f"d{x}_{j}")
                if j >= EMBUFS:
                    # WAR absorber: take the one recycled-slot wait on a tiny
                    # ACT op so the bulk exp keeps a single (DMA) wait.
                    old = dts_hist[j - EMBUFS][x]
                    nc.scalar.activation(
                        out=old[0:1, 0, 0:1], in_=old[0:1, 0, 0:1], func=COPY
                    )
                # d = exp(emit - c) for the whole chunk at once on ACT
                nc.scalar.activation(
                    out=dt[:, 0:kj, :], in_=raw[:, 0:kj, :],
                    func=EXP, bias=cv_t[:, 1:2], scale=1.0,
                )
                # Re-home the chunk on DVE: the per-step muls then read a
                # DVE-written tile, so their d-dep needs no sem waits.
                dd = empool.tile([128, 16, CB], bf16, tag=f"dd{x}", name=f"dd{x}_{j}")
                nc.vector.tensor_copy(dd[:, 0:kj, :], dt[:, 0:kj, :])
                dds.append((dt, dd))
            dts_hist.append([a for a, _ in dds])
            t0 += kj
            for k in range(kj):
                for x in range(NCHAIN):
                    d_sl = dds[x][1][:, k, :]
                    s_new = state.tile([128, CB], bf16, tag=f"s{x}", name=f"s{x}_{j}_{k}")
                    if j == 0 and k == 0:
                        # s_0 = [exp(start); exp(end)] * d_0
                        nc.vector.tensor_scalar_mul(s_new, d_sl, cv_t[:, 0:1])
                    else:
                        q = psum.tile([128, CB], f32, tag=f"q{x}", name=f"q{x}_{j}_{k}")
                        nc.tensor.matmul(q, w_t, s_cur[x], start=True, stop=True)
                        nc.vector.tensor_mul(s_new, q, d_sl)
                    s_cur[x] = s_new

        for x in range(NCHAIN):
            # one more combined matmul: top half = E^T p_255 (forward alpha)
            qf = psum.tile([128, CB], f32, tag=f"q{x}", name=f"qf{x}")
            nc.tensor.matmul(qf, w_t, s_cur[x], start=True, stop=True)
            # bring the backward half of the state (v_256) down to parts 0:64
            vs = psum.tile([64, CB], f32, tag=f"vs{x}", bufs=1)
            nc.tensor.matmul(vs, ish_t, s_cur[x], start=True, stop=True)
            vsb = state.tile([64, CB], f32, tag=f"vsb{x}")
            nc.vector.tensor_copy(vsb, vs)
            zz = state.tile([64, CB], f32, tag=f"zz{x}")
            nc.vector.tensor_mul(zz, qf[0:64, :], vsb)
            zs = psum.tile([1, CB], f32, tag="warm", bufs=1, name=f"zs{x}")
            nc.tensor.matmul(zs, on_t, zz, start=True, stop=True)
            res = state.tile([1, CB], f32, tag=f"res{x}")
            nc.scalar.activation(out=res, in_=zs, func=LN)
            nc.sync.dma_start(out=outp[x : x + 1, :], in_=res)
    nc.compile()
    _strip_redundant_ldweights(nc)
    return nc


def _strip_redundant_ldweights(nc):
    """Drop InstLdweights that reload the exact weights already resident in
    the PE array (bacc emits one per matmult; the step matmuls all reuse the
    same stationary tile). Generated LDWs carry no sem updates, so deletion
    does not shift semaphore counts. Only LDWs with empty waits/updates and
    a signature equal to the last kept LDW are removed."""
    for f in nc.m.functions:
        for b in f.blocks:
            il = b.instructions
            last_sig = None
            i = 0
            while i < len(il):
                ins = il[i]
                tn = type(ins).__name__
                if tn == 'InstLdweights':
                    si = ins.sync_info
                    clean = not (
                        (si and (list(si.on_wait) or list(si.on_update)))
                        or getattr(ins, 'is_transpose', None)
                        or getattr(ins, 'perf_mode', None)
                    )
                    sig = (
                        str(ins.ins[0]),
                        str(getattr(ins, 'tile_position', None)),
                    )
                    if clean and sig == last_sig:
                        del il[i]
                        continue
                    last_sig = sig
                elif tn == 'InstMatmult':
                    if getattr(ins, 'is_transpose', None):
                        last_sig = None  # transpose clobbers the array
                i += 1


def _prep_inputs(emissions, transitions, start_transitions, end_transitions):
    """Host-side: shard + transpose emissions, build tiny constant tensors."""
    em = np.ascontiguousarray(emissions, dtype=np.float32)
    T = np.asarray(transitions, dtype=np.float32)
    st = np.asarray(start_transitions, dtype=np.float32)
    en = np.asarray(end_transitions, dtype=np.float32)

    E = np.exp(T).astype(np.float32)
    wts = np.zeros((128, 128), dtype=ml_dtypes.bfloat16)
    wts[:64, :64] = E        # forward: q = E^T p (contract over partitions)
    wts[64:, 64:] = E.T      # backward: u = E v

    cvec = np.zeros((128, 2), dtype=np.float32)
    cvec[:64, 0] = np.exp(st)
    cvec[64:, 0] = np.exp(en)
    cvec[:, 1] = -C_NORM

    ish = np.zeros((128, 64), dtype=ml_dtypes.bfloat16)
    ish[64 + np.arange(64), np.arange(64)] = 1.0  # partition shift 64->0

    ones = np.ones((64, 1), dtype=np.float32)

    in_maps = []
    for i in range(NCORES):
        sl = em[i * BPC : (i + 1) * BPC]  # [128, 512, 64]
        chains = []
        for x in range(NCHAIN):
            half = sl[x * CB : (x + 1) * CB]             # [64, 512, 64] (b, t, l)
            fwd = half[:, :TM, :].transpose(1, 2, 0)      # [256, 64l, 64b]
            bwd = half[:, TM:, :][:, ::-1, :].transpose(1, 2, 0)  # t = 511-k
            comb = np.concatenate([fwd, bwd], axis=1)     # [256wf, 128p, 64b]
            chains.append(np.ascontiguousarray(comb.transpose(1, 0, 2)))  # [128, 256, 64]
        emt = np.ascontiguousarray(np.stack(chains))      # [2, 128, 256, 64]
        in_maps.append({"emt": emt, "wts": wts, "cvec": cvec, "ish": ish, "ones": ones})
    return in_maps


def _run(in_maps, trace=False, **kw):
    if "nc" not in _CACHE:
        _CACHE["nc"] = _build_nc()
    return run_bass_kernel_spmd(
        _CACHE["nc"], in_maps, core_ids=list(range(NCORES)), trace=trace, **kw
    )


def kernel(emissions, mask, transitions, start_transitions, end_transitions):
    # mask is all-ones for this problem (fill: "ones"); the masked update
    # reduces to the unmasked recurrence, so it is not used.
    in_maps = _prep_inputs(emissions, transitions, start_transitions, end_transitions)
    res = _run(in_maps)
    outs = np.stack([r["out"] for r in res.results])  # [8, 2, 64]
    return (outs.reshape(B) + np.float32(S * C_NORM)).astype(np.float32)


# revision 14
# speedup vs baseline: 1.0030x; 1.0030x over previous
"""CRF forward (log-partition) kernel for Trainium2, 8 NeuronCores.

Algorithm: exp-space scaled forward recurrence (classic scaled HMM forward),
split into a forward and a backward half that run simultaneously and meet in
the middle — this halves the serial dependency chain (the kernel is bound by
per-step PE<->DVE roundtrip latency, not throughput).

    forward : p_k = d_k * (E^T p_{k-1}),  p_0 = exp(start) * d_0
    backward: v_t = d_t * (E v_{t+1}),    v_511 = exp(end) * d_511
    d_t = exp(emit_t - c),  E = exp(T),  c = fixed rescale constant
    logZ = S*c + ln( sum_j (E^T p_255)_j * (v_256)_j )

Both directions share each matmul: the stationary weight is
blockdiag(E, E^T) [128x128]; the state tile stacks [64 forward labels |
64 backward labels] on partitions with batch on the free dim. bf16 state /
weights (safe: the output is log-scale ~2379, so ~0.2% linear-space rounding
averages out to ~3e-5 relative error).

Sharding: batch 1024 -> 8 cores x 128; per core 2 interleaved wavefront
chains (batch halves) hide the PE<->DVE latency. Emissions are
pre-transposed on the host into DMA-contiguous per-chunk tiles, exp'd in
bulk on ACT, then re-homed to DVE (so the per-step muls carry no
cross-engine waits). Redundant per-matmul LDWEIGHTS are stripped
post-compile (the stationary weights never change mid-loop).
"""

import numpy as np
import ml_dtypes
from contextlib import ExitStack

import concourse.bass as bass
import concourse.bacc as bacc
import concourse.tile as tile
from concourse import mybir
from concourse.bass_utils import run_bass_kernel_spmd

# Problem constants (hardcoded per contract: shapes are fixed)
B, S, L = 1024, 512, 64
NCORES = 8
NCHAIN = 2            # wavefront chains per core (batch halves)
BPC = B // NCORES     # 128 batch per core
CB = BPC // NCHAIN    # 64 batch per chain = matmul free dim
TM = S // 2           # 256 wavefronts (fwd + bwd meet in the middle)
# Variable chunk schedule: small head chunks so the recurrence starts while
# the bulk still streams; 16-wavefront chunks in steady state.
CHUNKS = [2, 2, 4, 8] + [16] * 15
assert sum(CHUNKS) == TM
C_NORM = 4.6466287    # per-step rescale constant (offline calibrated)

_CACHE: dict = {}


def _build_nc():
    f32 = mybir.dt.float32
    bf16 = mybir.dt.bfloat16
    nc = bacc.Bacc(None, target_bir_lowering=False)
    emt = nc.declare_dram_parameter(
        "emt", [NCHAIN, 128, TM, CB], f32, isOutput=False
    )
    wts = nc.declare_dram_parameter("wts", [128, 128], bf16, isOutput=False)
    cvec = nc.declare_dram_parameter("cvec", [128, 2], f32, isOutput=False)
    ish = nc.declare_dram_parameter("ish", [128, 64], bf16, isOutput=False)
    ones = nc.declare_dram_parameter("ones", [64, 1], f32, isOutput=False)
    outp = nc.declare_dram_parameter("out", [NCHAIN, CB], f32, isOutput=True)

    EXP = mybir.ActivationFunctionType.Exp
    LN = mybir.ActivationFunctionType.Ln
    COPY = mybir.ActivationFunctionType.Copy
    EMBUFS = 5

    with ExitStack() as ctx:
        tc = ctx.enter_context(tile.TileContext(nc))
        consts = ctx.enter_context(tc.tile_pool(name="consts", bufs=1))
        empool = ctx.enter_context(tc.tile_pool(name="em", bufs=EMBUFS))
        state = ctx.enter_context(tc.tile_pool(name="state", bufs=12))
        psum = ctx.enter_context(
            tc.tile_pool(name="psum", bufs=2, space=bass.MemorySpace.PSUM)
        )

        w_t = consts.tile([128, 128], bf16)
        cv_t = consts.tile([128, 2], f32)
        ish_t = consts.tile([128, 64], bf16)
        on_t = consts.tile([64, 1], f32)
        nc.sync.dma_start(out=w_t, in_=wts[:, :])
        nc.sync.dma_start(out=cv_t, in_=cvec[:, :])
        nc.sync.dma_start(out=ish_t, in_=ish[:, :])
        nc.sync.dma_start(out=on_t, in_=ones[:, :])

        # Warmups: make each engine observe the const DMAs up front so no
        # steady-state instruction needs more than one sem wait.
        aw = consts.tile([128, 2], f32, tag="actwarm")
        nc.scalar.activation(out=aw, in_=cv_t, func=COPY)
        dw = consts.tile([128, 1], f32, tag="dvewarm")
        nc.vector.tensor_copy(dw, cv_t[:, 0:1])
        ow = consts.tile([64, 1], f32, tag="oneswarm")
        nc.vector.tensor_copy(ow, on_t)
        wq = psum.tile([128, 2], f32, tag="warm", bufs=1)
        nc.tensor.matmul(wq[0:64, :], ish_t[:, 0:64], ish_t[:, 0:2], start=True, stop=True)
        # last warmup leaves the main stationary weights resident
        nc.tensor.matmul(wq, w_t, ish_t[:, 0:2], start=True, stop=True)

        s_cur = [None] * NCHAIN
        dts_hist: list[list] = []
        t0 = 0
        for j, kj in enumerate(CHUNKS):
            dds = []
            for x in range(NCHAIN):
                raw = empool.tile([128, 16, CB], f32, tag=f"raw{x}", name=f"raw{x}_{j}")
                # split chains across the two HWDGE rings (SP and ACT) so
                # chunk DMAs issue in parallel
                dma_eng = nc.sync if x == 0 else nc.scalar
                dma_eng.dma_start(
                    out=raw[:, 0:kj, :], in_=emt[x, :, t0 : t0 + kj, :]
                )
                dt = empool.tile([128, 16, CB], bf16, tag=f"d{x}", name=f"d{x}_{j}")
                if j >= EMBUFS:
                    # WAR absorber: take the one recycled-slot wait on a tiny
                    # ACT op so the bulk exp keeps a single (DMA) wait.
                    old = dts_hist[j - EMBUFS][x]
                    nc.scalar.activation(
                        out=old[0:1, 0, 0:1], in_=old[0:1, 0, 0:1], func=COPY
                    )
                # d = exp(emit - c) for the whole chunk at once on ACT
                nc.scalar.activation(
                    out=dt[:, 0:kj, :], in_=raw[:, 0:kj, :],
                    func=EXP, bias=cv_t[:, 1:2], scale=1.0,
                )
                # Re-home the chunk on DVE: the per-step muls then read a
                # DVE-written tile, so their d-dep needs no sem waits.
                dd = empool.tile([128, 16, CB], bf16, tag=f"dd{x}", name=f"dd{x}_{j}")
                nc.vector.tensor_copy(dd[:, 0:kj, :], dt[:, 0:kj, :])
                dds.append((dt, dd))
            dts_hist.append([a for a, _ in dds])
            t0 += kj
            for k in range(kj):
                for x in range(NCHAIN):
                    d_sl = dds[x][1][:, k, :]
                    s_new = state.tile([128, CB], bf16, tag=f"s{x}", name=f"s{x}_{j}_{k}")
                    if j == 0 and k == 0:
                        # s_0 = [exp(start); exp(end)] * d_0
                        nc.vector.tensor_scalar_mul(s_new, d_sl, cv_t[:, 0:1])
                    else:
                        q = psum.tile([128, CB], f32, tag=f"q{x}", name=f"q{x}_{j}_{k}")
                        nc.tensor.matmul(q, w_t, s_cur[x], start=True, stop=True)
                        nc.vector.tensor_mul(s_new, q, d_sl)
                    s_cur[x] = s_new

        for x in range(NCHAIN):
            # one more combined matmul: top half = E^T p_255 (forward alpha)
            qf = psum.tile([128, CB], f32, tag=f"q{x}", name=f"qf{x}")
            nc.tensor.matmul(qf, w_t, s_cur[x], start=True, stop=True)
            # bring the backward half of the state (v_256) down to parts 0:64
            vs = psum.tile([64, CB], f32, tag=f"vs{x}", bufs=1)
            nc.tensor.matmul(vs, ish_t, s_cur[x], start=True, stop=True)
            vsb = state.tile([64, CB], f32, tag=f"vsb{x}")
            nc.vector.tensor_copy(vsb, vs)
            zz = state.tile([64, CB], f32, tag=f"zz{x}")
            nc.vector.tensor_mul(zz, qf[0:64, :], vsb)
            zs = psum.tile([1, CB], f32, tag="warm", bufs=1, name=f"zs{x}")
            nc.tensor.matmul(zs, on_t, zz, start=True, stop=True)
            res = state.tile([1, CB], f32, tag=f"res{x}")
            nc.scalar.activation(out=res, in_=zs, func=LN)
            nc.sync.dma_start(out=outp[x : x + 1, :], in_=res)
    nc.compile()
    _strip_redundant_ldweights(nc)
    return nc


def _strip_redundant_ldweights(nc):
    """Drop InstLdweights that reload the exact weights already resident in
    the PE array (bacc emits one per matmult; the step matmuls all reuse the
    same stationary tile). Generated LDWs carry no sem updates, so deletion
    does not shift semaphore counts. Only LDWs with empty waits/updates and
    a signature equal to the last kept LDW are removed."""
    for f in nc.m.functions:
        for b in f.blocks:
            il = b.instructions
            last_sig = None
            i = 0
            while i < len(il):
                ins = il[i]
                tn = type(ins).__name__
                if tn == 'InstLdweights':
                    si = ins.sync_info
                    clean = not (
                        (si and (list(si.on_wait) or list(si.on_update)))
                        or getattr(ins, 'is_transpose', None)
                        or getattr(ins, 'perf_mode', None)
                    )
                    sig = (
                        str(ins.ins[0]),
                        str(getattr(ins, 'tile_position', None)),
                    )
                    if clean and sig == last_sig:
                        del il[i]
                        continue
                    last_sig = sig
                elif tn == 'InstMatmult':
                    if getattr(ins, 'is_transpose', None):
                        last_sig = None  # transpose clobbers the array
                i += 1


def _prep_inputs(emissions, transitions, start_transitions, end_transitions):
    """Host-side: shard + transpose emissions, build tiny constant tensors."""
    em = np.ascontiguousarray(emissions, dtype=np.float32)
    T = np.asarray(transitions, dtype=np.float32)
    st = np.asarray(start_transitions, dtype=np.float32)
    en = np.asarray(end_transitions, dtype=np.float32)

    E = np.exp(T).astype(np.float32)
    wts = np.zeros((128, 128), dtype=ml_dtypes.bfloat16)
    wts[:64, :64] = E        # forward: q = E^T p (contract over partitions)
    wts[64:, 64:] = E.T      # backward: u = E v

    cvec = np.zeros((128, 2), dtype=np.float32)
    cvec[:64, 0] = np.exp(st)
    cvec[64:, 0] = np.exp(en)
    cvec[:, 1] = -C_NORM

    ish = np.zeros((128, 64), dtype=ml_dtypes.bfloat16)
    ish[64 + np.arange(64), np.arange(64)] = 1.0  # partition shift 64->0

    ones = np.ones((64, 1), dtype=np.float32)

    in_maps = []
    for i in range(NCORES):
        sl = em[i * BPC : (i + 1) * BPC]  # [128, 512, 64]
        chains = []
        for x in range(NCHAIN):
            half = sl[x * CB : (x + 1) * CB]             # [64, 512, 64] (b, t, l)
            fwd = half[:, :TM, :].transpose(1, 2, 0)      # [256, 64l, 64b]
            bwd = half[:, TM:, :][:, ::-1, :].transpose(1, 2, 0)  # t = 511-k
            comb = np.concatenate([fwd, bwd], axis=1)     # [256wf, 128p, 64b]
            chains.append(np.ascontiguousarray(comb.transpose(1, 0, 2)))  # [128, 256, 64]
        emt = np.ascontiguousarray(np.stack(chains))      # [2, 128, 256, 64]
        in_maps.append({"emt": emt, "wts": wts, "cvec": cvec, "ish": ish, "ones": ones})
    return in_maps


def _run(in_maps, trace=False, **kw):
    if "nc" not in _CACHE:
        _CACHE["nc"] = _build_nc()
    return run_bass_kernel_spmd(
        _CACHE["nc"], in_maps, core_ids=list(range(NCORES)), trace=trace, **kw
    )


def kernel(emissions, mask, transitions, start_transitions, end_transitions):
    # mask is all-ones for this problem (fill: "ones"); the masked update
    # reduces to the unmasked recurrence, so it is not used.
    in_maps = _prep_inputs(emissions, transitions, start_transitions, end_transitions)
    res = _run(in_maps)
    outs = np.stack([r["out"] for r in res.results])  # [8, 2, 64]
    return (outs.reshape(B) + np.float32(S * C_NORM)).astype(np.float32)
